# revision 32
# baseline (speedup 1.0000x reference)
"""BatchGAT (2-layer, 8-head GAT over 32 graphs of 512 nodes) on 8 TRN2 NeuronCores.

Data-parallel over the batch: each core processes 4 graphs, software-pipelined
over 8 (graph, layer) units. Per unit the masked-softmax attention E^T[j,i] is
built per head and aggregated with a TRANSPOSED matmul (lhsT = [hp | c-dup]
stationary, rhs = E^T moving) so each head costs 4 LDW + 4 wide MMs and the
output lands feature-major.

Head paths (softmax is invariant to per-column scaling of E; the dropped
exp(0.2 s_i) factor cancels in the softmax):
 'f' fused-DVE: ONE custom DVE op per (h, jt) [runtime-registered]:
     et = max(qbc*rsc, ed2) * adjP  with qbc = exp(0.8 s) broadcast,
     rsc = exp(d), ed2 = exp(0.2 d) per-partition scalars.
 't'/'g': dual-op TensorScalar (mult rsc, max ed2 - one DVE pass) then the
     adjP mask multiply as one full-width DVE TT ('t') or per-jt gpsimd TT
     ('g'; short Pool bursts - long ones starve concurrent DVE reads).
 'a' ACT/PE: logits s_i + adjM[j,i] via PE (K=1 ones matmul + identity
     accumulate), d_j added as the per-partition Prelu bias; et = exp(Prelu).

Normalization: c-dup lhsT columns replicate c*den across 64 PSUM rows (even
heads rows 64-127, odd heads use a flipped lhsT so den lands at rows 0-63).
Per wave: odd den reciprocal runs PSUM-direct on DVE; even den is re-based
to partitions 0-63 by one ACT copy first (DVE dst/src0 partition bases must
match; only src1 may differ - probed). Norm TT mults write x1 during PSUM
evacuation. L0 tail per wave: ACT Exp + ONE custom DVE op
x1 = min(relu(x), e^x - 1). Layer-1 folds the head-mean via c=8; po and den
ship to the host which finishes x2 = po/den and the wave/row sums.

DMA queues: bulk input loads ride the sync queue in per-jt chunks; the
qs->qd DRAM bounce issues from ACT, the qbc partition-broadcast from gpsimd
in 2-head chunks (the broadcast transfer is the unit-start latency), out2
from sync - blocking semaphore waits on a queue serialize every later DMA
issued from it, so producer-matched queues keep them wait-free.
"""

import os
import sys

if "/opt/trn_rl_repo" not in sys.path:
    sys.path.insert(0, "/opt/trn_rl_repo")

import numpy as np
import ml_dtypes

import concourse.bacc as bacc
import concourse.mybir as mybir
from concourse import tile
from concourse.bass_utils import run_bass_kernel_spmd
from concourse.alu_op_type import AluOpType

F32 = mybir.dt.float32
BF16 = mybir.dt.bfloat16
BF = ml_dtypes.bfloat16
AF = mybir.ActivationFunctionType

B, N, FIN, H, F = 32, 512, 64, 8, 64
NCORES = 8
G = B // NCORES          # graphs per core
NT = N // 128            # node tiles
C1 = H * F               # layer-1 input features (512)
BIG = 30000.0

# ---- load-balance knobs ----
# per-head path chars (index = head):
#  'a' ACT/PE path (PE logits + Prelu + Exp)
#  'f' fused custom DVE (1 op/tile)
#  't' TS-dual DVE + full-width mask TT on DVE
#  'g' TS-dual DVE + full-width mask TT on gpsimd
PATH0 = os.environ.get("GAT_P0", "afatgfta")
PATH1 = os.environ.get("GAT_P1", "afgtgata")
NORM_ENG = os.environ.get("GAT_NORM", "vector")   # vector | gpsimd | mix
ELU_ENG = os.environ.get("GAT_ELU", "vector")     # vector | split
EV_ENG = os.environ.get("GAT_EV", "act")        # act | vector | split
N_UNITS = int(os.environ.get("GAT_UNITS", "8"))
LOOK = int(os.environ.get("GAT_LOOK", "2"))
ET_BUFS = int(os.environ.get("GAT_ETB", "5"))
LR_BUFS = int(os.environ.get("GAT_LRB", "4"))
AGG_MODE = os.environ.get("GAT_AGG", "wave")      # head | wave
AGG_BUFS = int(os.environ.get("GAT_AGB", "5"))

_cached = {}
_OPS = {}


def _register_custom_ops():
    """Register the two fused DVE ops via the documented extension point
    (dve_ops.OPS); idempotent, sha computed at runtime."""
    if _OPS:
        return
    import concourse.dve_ops as dve_ops
    from concourse.dve_spec import (
        Spec, Src0, Src1, C0, C1 as SC1, maxx, minn, relu, lower,
        _has_src1,
    )
    from concourse.dve_uop import DveOpSpec

    have = {op.name for op in dve_ops.OPS}

    def mk(name, spec):
        if name in have:
            _OPS[name] = next(o for o in dve_ops.OPS if o.name == name)
            return
        row = dve_ops._CUSTOM_DVE_ROW_BASE + len(dve_ops.OPS)
        shas = {}
        for ver in ("v3", "v4"):
            uops = lower(spec, ver=ver)
            shas[ver] = DveOpSpec(
                name=name, opcode=row, uops=uops, rd1_en=_has_src1(spec)
            ).sha(ver)
        op = dve_ops.DveOp(name, spec, subdim=False, uops_sha=shas)
        dve_ops.OPS.append(op)
        dve_ops._SUB_OPCODE_FOR_NAME[name] = row
        dve_ops.CUSTOM_DVE_SPECS[name] = spec
        _OPS[name] = op

    mk("ATT_MASK_FUSED_GAT", Spec(
        body=maxx(Src0 * C0, SC1) * Src1,
        reference=lambda in0, in1, s0, s1, imm2: (
            np.maximum(in0.astype(np.float32) * s0, s1) * in1
        ).astype(np.float32),
    ))
    mk("ELU_TAIL_GAT", Spec(
        body=minn(relu(Src0), Src1 - C0),
        reference=lambda in0, in1, s0, s1, imm2: np.minimum(
            np.maximum(np.nan_to_num(in0.astype(np.float32), nan=0.0), 0.0),
            in1.astype(np.float32) - s0,
        ).astype(np.float32),
    ))


def _build():
    _register_custom_ops()
    nc = bacc.Bacc("TRN2", target_bir_lowering=False, debug=False)

    need_adjm = ("a" in PATH0) or ("a" in PATH1)
    xtb = nc.dram_tensor("xtb", [G, FIN, N], BF16, kind="ExternalInput").ap()
    adjP = nc.dram_tensor("adjP", [G, N, N], BF16, kind="ExternalInput").ap()
    adjM = nc.dram_tensor("adjM", [G, N, N], BF16, kind="ExternalInput").ap()
    ident = nc.dram_tensor("ident", [128, 128], BF16, kind="ExternalInput").ap()
    w0d = nc.dram_tensor("w0d", [FIN, F + 2 * H], BF16, kind="ExternalInput").ap()
    w1d = nc.dram_tensor("w1d", [C1, F + 2 * H], BF16, kind="ExternalInput").ap()
    out2 = nc.dram_tensor("out2", [G, 4, 65, 2 * N], F32,
                          kind="ExternalOutput").ap()

    with tile.TileContext(nc) as tc:
        _emit(nc, tc, xtb, adjP, adjM, ident, w0d, w1d, out2, need_adjm)
    nc.compile()
    return nc


def _emit(nc, tc, xtb, adjP, adjM, ident, w0d, w1d, out2, need_adjm):
    from contextlib import ExitStack

    att_op = _OPS["ATT_MASK_FUSED_GAT"]
    elu_op = _OPS["ELU_TAIL_GAT"]

    ctx = ExitStack()
    with ctx:
        wpool = ctx.enter_context(tc.tile_pool(name="weights", bufs=1))
        w0d_sb = wpool.tile([FIN, F + 2 * H], BF16, tag="w0d")
        nc.sync.dma_start(w0d_sb[:], w0d[:])
        w1d_sb = wpool.tile([128, NT, F + 2 * H], BF16, tag="w1d")
        nc.sync.dma_start(w1d_sb[:], w1d.rearrange("(c p) f -> p c f", p=128))
        id_sb = wpool.tile([128, 128], BF16, tag="ident")
        nc.sync.dma_start(id_sb[:], ident[:])
        ones1 = wpool.tile([1, 128], BF16, tag="ones1")
        nc.vector.memset(ones1[:], 1.0)

        xt_pool = ctx.enter_context(tc.tile_pool(name="xt", bufs=3))
        adj_pool = ctx.enter_context(
            tc.tile_pool(name="adj", bufs=4 if need_adjm else 2))
        row_pool = ctx.enter_context(tc.tile_pool(name="rows", bufs=3))
        sc_pool = ctx.enter_context(tc.tile_pool(name="scal", bufs=3))
        ha_pool = ctx.enter_context(tc.tile_pool(name="ha", bufs=3))
        qbc_pool = ctx.enter_context(tc.tile_pool(name="qbc", bufs=3))
        et_pool = ctx.enter_context(tc.tile_pool(name="et", bufs=ET_BUFS))
        lr_pool = ctx.enter_context(tc.tile_pool(name="lr", bufs=LR_BUFS))
        rdb_pool = ctx.enter_context(tc.tile_pool(name="rdb", bufs=2))
        x1t_pool = ctx.enter_context(tc.tile_pool(name="x1t", bufs=3))
        post_pool = ctx.enter_context(tc.tile_pool(name="post", bufs=2))
        out_pool = ctx.enter_context(tc.tile_pool(name="outs", bufs=2))
        qd_pool = ctx.enter_context(tc.tile_pool(name="qd", bufs=3, space="DRAM"))

        ps_agg = ctx.enter_context(tc.tile_pool(
            name="ps_agg", bufs=AGG_BUFS if AGG_MODE == "head" else 2,
            space="PSUM"))
        ps_pp = ctx.enter_context(tc.tile_pool(name="ps_pp", bufs=1, space="PSUM"))
        ps_sd = ctx.enter_context(tc.tile_pool(name="ps_sd", bufs=1, space="PSUM"))
        ps_lg = ctx.enter_context(tc.tile_pool(
            name="ps_lg", bufs=1 if AGG_MODE == "head" else 2, space="PSUM"))

        graphs = {}

        def prologue(g, layer):
            """Projection + row/scalar extraction + broadcasts for one unit."""
            st = {}
            path = PATH0 if layer == 0 else PATH1
            if layer == 0:
                # inputs ride the tensor queue (no waits; frees q1 for qbc)
                xt = xt_pool.tile([FIN, N], BF16, tag="xt", name=f"xt_{g}")
                nc.sync.dma_start(xt[:], xtb[g])
                # per-jt chunk DMAs parallelize across DMA engines (a single
                # 512KB transfer rides one ~22.5GB/s engine for ~23us)
                ap_ = adj_pool.tile([128, NT * N], BF16, tag="adjp",
                                    name=f"adjp_{g}")
                apv = adjP[g].rearrange("(j p) i -> p j i", p=128)
                for jt in range(NT):
                    nc.sync.dma_start(ap_[:, jt * N:(jt + 1) * N],
                                      apv[:, jt])
                gd = dict(xt=xt, adjp=ap_)
                if need_adjm:
                    am_ = adj_pool.tile([128, NT * N], BF16, tag="adjm",
                                        name=f"adjm_{g}")
                    amv = adjM[g].rearrange("(j p) i -> p j i", p=128)
                    for jt in range(NT):
                        nc.sync.dma_start(am_[:, jt * N:(jt + 1) * N],
                                          amv[:, jt])
                    gd["adjm"] = am_
                graphs[g] = gd
            gs = graphs[g]
            xt, x1t = gs["xt"], gs.get("x1t")

            # s, d row vectors first: they gate the qs->qd->qbc DMA chain
            psd = ps_sd.tile([2 * H, N], F32, tag="sd", name=f"psd_{g}_{layer}")
            if layer == 0:
                nc.tensor.matmul(psd[:], w0d_sb[:, F:], xt[:],
                                 start=True, stop=True)
            else:
                for ct in range(NT):
                    nc.tensor.matmul(psd[:], w1d_sb[:, ct, F:],
                                     x1t[:, ct * N:(ct + 1) * N],
                                     start=(ct == 0), stop=(ct == NT - 1))

            # ---- projections: pp_all[j, jt*80: [hp(64) | d(8) | s(8)]] ----
            pp = ps_pp.tile([128, NT * (F + 2 * H)], F32, tag="pp",
                            name=f"pp_{g}_{layer}")
            W = F + 2 * H
            for jt in range(NT):
                dst = pp[:, jt * W:(jt + 1) * W]
                if layer == 0:
                    nc.tensor.matmul(dst, xt[:, jt * 128:(jt + 1) * 128],
                                     w0d_sb[:], start=True, stop=True)
                else:
                    for ct in range(NT):
                        nc.tensor.matmul(
                            dst, x1t[:, ct * N + jt * 128:ct * N + (jt + 1) * 128],
                            w1d_sb[:, ct, :], start=(ct == 0), stop=(ct == NT - 1))

            # ---- ACT extractions (read all 16 psd rows: ACT srcs must be
            # 32-aligned; rows 0-7 are d-junk, the DMA slices rows 8-15) ----
            qs = row_pool.tile([2 * H, 2 * N], BF16, tag="qs",
                               name=f"qs_{g}_{layer}")
            nc.scalar.activation(qs[:, 0:N], psd[:], AF.Exp, scale=0.8)
            if "a" in path:
                nc.scalar.activation(qs[:, N:2 * N], psd[:], AF.Copy)
            ppv = pp[:].rearrange("p (j c) -> p j c", j=NT, c=W)
            # per-partition scalars from the d columns: r=exp(d), ed2=exp(.2 d)
            sc = sc_pool.tile([128, 3 * NT * H], F32, tag="sc",
                              name=f"sc_{g}_{layer}")
            scv = sc[:].rearrange("p (k j h) -> p k j h", k=3, j=NT)
            nc.scalar.activation(scv[:, 0], ppv[:, :, F:F + H], AF.Exp)
            nc.scalar.activation(scv[:, 1], ppv[:, :, F:F + H], AF.Exp, scale=0.2)
            if "a" in path:
                nc.scalar.activation(scv[:, 2], ppv[:, :, F:F + H], AF.Copy)
            rsc = scv[:, 0]     # [128, jt, h]
            ed2 = scv[:, 1]
            dcol = scv[:, 2]

            # lhsT tiles: ha = [hp | c-dup]; haf = [c-dup | hp] (layer 0 only,
            # for odd heads so their po lands at rows 64-127)  (c=1 or 8)
            cval = 1.0 if layer == 0 else 8.0
            ha = ha_pool.tile([128, NT * 128], BF16, tag="ha",
                              name=f"ha_{g}_{layer}")
            hav = ha[:].rearrange("p (j c) -> p j c", j=NT)
            nc.scalar.activation(hav[:, :, 0:F], ppv[:, :, 0:F], AF.Copy)
            nc.vector.memset(hav[:, :, F:128], cval)
            haf = None
            if layer == 0:
                haf = ha_pool.tile([128, NT * 128], BF16, tag="haf",
                                   name=f"haf_{g}_{layer}")
                hfv = haf[:].rearrange("p (j c) -> p j c", j=NT)
                nc.scalar.activation(hfv[:, :, F:128], ppv[:, :, 0:F], AF.Copy)
                nc.vector.memset(hfv[:, :, 0:F], cval)

            # q broadcast via DRAM bounce; s gathered flat for A-path matmuls.
            # Issue these on scalar/gpsimd queues: their semaphore waits would
            # otherwise serialize all later input loads behind the bounce.
            qd = qd_pool.tile([H, 2 * N], BF16, tag="qd", name=f"qd_{g}_{layer}")
            nc.scalar.dma_start(qd[:], qs[H:2 * H, :])
            qbc = qbc_pool.tile([128, H * N], BF16, tag="qbc",
                                name=f"qbc_{g}_{layer}")
            qbcv = qbc[:].rearrange("p (h i) -> p h i", h=H)
            for hc in range(0, H, 2):
                nc.gpsimd.dma_start(
                    qbcv[:, hc:hc + 2],
                    qd[hc:hc + 2, 0:N].unsqueeze(0).partition_broadcast(128))
            sfl = None
            if "a" in path:
                sfl = row_pool.tile([1, H * N], BF16, tag="sfl",
                                    name=f"sfl_{g}_{layer}")
                nc.gpsimd.dma_start(sfl[:], qd[:, N:2 * N])

            st.update(ha=ha, haf=haf, qbc=qbc, sfl=sfl, rsc=rsc, ed2=ed2,
                      dcol=dcol)
            return st

        def main(g, layer, st):
            gs = graphs[g]
            path = PATH0 if layer == 0 else PATH1
            adjp, adjm = gs["adjp"], gs.get("adjm")
            ha, haf, qbc, sfl = st["ha"], st["haf"], st["qbc"], st["sfl"]
            rsc, ed2, dcol = st["rsc"], st["ed2"], st["dcol"]

            if layer == 0:
                x1t = x1t_pool.tile([128, NT * N], BF16, tag="x1t",
                                    name=f"x1t_{g}")
                graphs[g]["x1t"] = x1t
                dstn = x1t_pool.tile([128, NT * N], BF16, tag="x1n",
                                     name=f"x1n_{g}")
            else:
                dstn = None

            for w in range(4):           # four waves of 2 heads
                if AGG_MODE != "head":
                    agg = ps_agg.tile([128, 2 * N], F32, tag="agg",
                                      name=f"agg_{g}_{layer}_{w}")
                for hh in range(2):
                    h = 2 * w + hh
                    p = path[h]
                    if AGG_MODE == "head":
                        aggh = ps_agg.tile([128, N], F32, tag="agg",
                                           name=f"agg_{g}_{layer}_{h}")
                    et = et_pool.tile([128, NT * N], BF16, tag="et",
                                      name=f"et_{g}_{layer}_{h}")
                    if p == "a":
                        lr = lr_pool.tile([128, NT * N], BF16, tag="lr",
                                          name=f"lr_{g}_{layer}_{h}")
                        for jt in range(NT):
                            lg = ps_lg.tile([128, N], F32, tag="lg",
                                            name=f"lg_{g}_{layer}_{h}_{jt}")
                            nc.tensor.matmul(
                                lg[:], ones1[:], sfl[:, h * N:(h + 1) * N],
                                start=True, stop=False)
                            nc.tensor.matmul(
                                lg[:], id_sb[:],
                                adjm[:, jt * N:(jt + 1) * N],
                                start=False, stop=True)
                            nc.scalar.activation(
                                lr[:, jt * N:(jt + 1) * N], lg[:], AF.Prelu,
                                bias=dcol[:, jt, h:h + 1], alpha=0.2)
                            if jt % 2 == 1:
                                # 2-jt Exp chunks unlock agg MMs earlier
                                sl = slice((jt - 1) * N, (jt + 1) * N)
                                nc.scalar.activation(et[:, sl], lr[:, sl],
                                                     AF.Exp)
                    elif p == "f":
                        for jt in range(NT):
                            nc.vector._custom_dve(
                                att_op,
                                out=et[:, jt * N:(jt + 1) * N],
                                in0=qbc[:, h * N:(h + 1) * N],
                                in1=adjp[:, jt * N:(jt + 1) * N],
                                s0=rsc[:, jt, h:h + 1],
                                s1=ed2[:, jt, h:h + 1])
                    else:
                        w_ = lr_pool.tile([128, NT * N], BF16, tag="lr",
                                          name=f"w_{g}_{layer}_{h}")
                        for jt in range(NT):
                            nc.vector.tensor_scalar(
                                w_[:, jt * N:(jt + 1) * N],
                                qbc[:, h * N:(h + 1) * N],
                                rsc[:, jt, h:h + 1], ed2[:, jt, h:h + 1],
                                AluOpType.mult, AluOpType.max)
                        if p == "g":
                            # per-jt chunks: short gpsimd bursts contend far
                            # less with concurrent DVE work than full-width
                            for jt in range(NT):
                                nc.gpsimd.tensor_tensor(
                                    et[:, jt * N:(jt + 1) * N],
                                    w_[:, jt * N:(jt + 1) * N],
                                    adjp[:, jt * N:(jt + 1) * N],
                                    AluOpType.mult)
                        else:
                            nc.vector.tensor_tensor(et[:], w_[:], adjp[:],
                                                    AluOpType.mult)
                    lhs = ha if (layer == 1 or h % 2 == 0) else haf
                    adst = aggh[:] if AGG_MODE == "head" else \
                        agg[:, hh * N:(hh + 1) * N]
                    for jt in range(NT):
                        nc.tensor.matmul(
                            adst,
                            lhs[:, jt * 128:(jt + 1) * 128],
                            et[:, jt * N:(jt + 1) * N],
                            start=(jt == 0), stop=(jt == NT - 1))

                    # ---- per-head evacuation (head mode) ----
                    if AGG_MODE != "head":
                        continue
                    if layer == 0:
                        dv = dstn[:, w * N:(w + 1) * N]
                        rdh = rdb_pool.tile([64, N], F32, tag="rdb",
                                            name=f"rd_{g}_{layer}_{h}")
                        if h % 2 == 0:
                            # ha: po rows 0-63, den rows 64-127 (ACT re-base)
                            dnE = rdb_pool.tile([64, N], F32, tag="dnE",
                                                name=f"dnE_{g}_{layer}_{h}")
                            nc.scalar.activation(dnE[:], aggh[64:128, :],
                                                 AF.Copy)
                            nc.vector.reciprocal_approx_fast(rdh[:], dnE[:])
                            nc.vector.tensor_tensor(dv[0:64], aggh[0:64, :],
                                                    rdh[:], AluOpType.mult)
                        else:
                            # haf: den rows 0-63 (PSUM-direct), po rows 64-127
                            nc.vector.reciprocal_approx_fast(rdh[:],
                                                             aggh[0:64, :])
                            nc.vector.tensor_tensor(dv[64:128],
                                                    aggh[64:128, :],
                                                    rdh[:], AluOpType.mult)
                    else:
                        ev = out_pool.tile([65, N], F32, tag="ev",
                                           name=f"ev_{g}_{h}")
                        if EV_ENG == "act" or (EV_ENG == "split"
                                               and h % 2 == 0):
                            nc.scalar.activation(ev[:], aggh[0:65, :],
                                                 AF.Copy)
                        else:
                            nc.vector.tensor_copy(ev[:], aggh[0:65, :])
                        nc.sync.dma_start(
                            out2[g, w, :, hh * N:(hh + 1) * N], ev[:])

                # ---- evacuate wave (wave mode) ----
                if AGG_MODE == "head":
                    continue
                if layer == 0:
                    # dens: odd head rows 0-63 cols N:2N (PSUM-direct recip);
                    # even head rows 64-127 cols 0:N (ACT copy re-bases to
                    # partitions 0-63, then recip). DVE partition rule: dst
                    # and src0 bases match; src1 base may sit lower (proven).
                    rd = rdb_pool.tile([64, 2 * N], F32, tag="rdb",
                                       name=f"rdb_{g}_{layer}_{w}")
                    dnE = rdb_pool.tile([64, N], F32, tag="dnE",
                                        name=f"dnE_{g}_{layer}_{w}")
                    nc.scalar.activation(dnE[:], agg[64:128, 0:N], AF.Copy)
                    nc.vector.reciprocal_approx_fast(rd[:, 0:N],
                                                     agg[0:64, N:2 * N])
                    nc.vector.reciprocal_approx_fast(rd[:, N:2 * N], dnE[:])
                    dv = dstn[:, w * N:(w + 1) * N]
                    ne = (nc.gpsimd if NORM_ENG == "gpsimd" else nc.vector)
                    ne2 = (nc.gpsimd if NORM_ENG in ("gpsimd", "mix")
                           else nc.vector)
                    ne.tensor_tensor(dv[0:64], agg[0:64, 0:N],
                                     rd[:, N:2 * N], AluOpType.mult)
                    ne2.tensor_tensor(dv[64:128], agg[64:128, N:2 * N],
                                      rd[:, 0:N], AluOpType.mult)
                else:
                    # L1: ship po rows 0-63 + den row 64 out; host normalizes
                    ev = out_pool.tile([65, 2 * N], F32, tag="ev",
                                       name=f"ev_{g}_{w}")
                    if EV_ENG == "act":
                        nc.scalar.activation(ev[:], agg[0:65, :], AF.Copy)
                    elif EV_ENG == "vector":
                        nc.vector.tensor_copy(ev[:], agg[0:65, :])
                    else:
                        nc.scalar.activation(ev[:, 0:N], agg[0:65, 0:N],
                                             AF.Copy)
                        nc.vector.tensor_copy(ev[:, N:2 * N],
                                              agg[0:65, N:2 * N])
                    nc.sync.dma_start(out2[g, w, :, 0:N], ev[:, 0:N])
                    nc.sync.dma_start(out2[g, w, :, N:2 * N], ev[:, N:2 * N])

            if layer == 0:
                # per-wave elu tail: elu(x) = min(relu(x), exp(x) - 1);
                # chunked so x1t columns unlock as each wave's dv lands
                x1t = graphs[g]["x1t"]
                expt = post_pool.tile([128, NT * N], BF16, tag="expt",
                                      name=f"expt_{g}")
                for w in range(4):
                    sl = slice(w * N, (w + 1) * N)
                    nc.scalar.activation(expt[:, sl], dstn[:, sl], AF.Exp)
                    nc.vector._custom_dve(elu_op, out=x1t[:, sl],
                                          in0=dstn[:, sl],
                                          in1=expt[:, sl], s0=1.0)

        # software-pipelined unit order: prologue of unit k+1 lands before
        # main of unit k
        if os.environ.get("GAT_ORDER", "mix") == "l0first":
            U = [(0, 0), (1, 0), (2, 0), (3, 0), (0, 1), (1, 1), (2, 1), (3, 1)]
        else:
            U = [(0, 0), (1, 0), (2, 0), (0, 1), (3, 0), (1, 1), (2, 1), (3, 1)]
        U = U[:N_UNITS]
        pending = {}
        for k in range(min(LOOK, len(U))):
            pending[U[k]] = prologue(*U[k])
        for i, u in enumerate(U):
            main(u[0], u[1], pending.pop(u))
            if i + LOOK < len(U):
                nxt = U[i + LOOK]
                pending[nxt] = prologue(*nxt)
        if not any(l == 1 for _, l in U):
            # bisect mode: make sure out2 is written so fetch succeeds
            z = out_pool.tile([65, 2 * N], F32, tag="ev", name="zz")
            nc.vector.memset(z[:], 0.0)
            for g in range(G):
                for w in range(4):
                    nc.sync.dma_start(out2[g, w], z[:])


def _get_nc():
    if "nc" not in _cached:
        _cached["nc"] = _build()
    return _cached["nc"]


def _prep_inputs(x, adj, W0, a_src0, a_dst0, W1, a_src1, a_dst1):
    x = np.asarray(x, np.float32)
    adj = np.array(adj, np.float32, copy=True)
    idx = np.arange(N)
    adj[:, idx, idx] = 1.0  # self loops (reference mutates adj the same way)
    xT = np.ascontiguousarray(x.transpose(0, 2, 1)).astype(BF)  # [B, 64, 512]
    adjPf = np.where(adj > 0, np.float32(1), np.float32(0)).astype(BF)
    adjMf = np.where(adj > 0, np.float32(0), np.float32(-BIG)).astype(BF)
    identf = np.eye(128, dtype=np.float32).astype(BF)
    W0 = np.asarray(W0, np.float32)
    W1 = np.asarray(W1, np.float32)
    w0d = np.concatenate(
        [W0, W0 @ np.asarray(a_dst0, np.float32),
         W0 @ np.asarray(a_src0, np.float32)], axis=1).astype(BF)
    w1d = np.concatenate(
        [W1, W1 @ np.asarray(a_dst1, np.float32),
         W1 @ np.asarray(a_src1, np.float32)], axis=1).astype(BF)
    in_maps = []
    for c in range(NCORES):
        sl = slice(c * G, (c + 1) * G)
        in_maps.append(dict(
            xtb=np.ascontiguousarray(xT[sl]),
            adjP=np.ascontiguousarray(adjPf[sl]),
            adjM=np.ascontiguousarray(adjMf[sl]),
            ident=identf, w0d=w0d, w1d=w1d,
        ))
    return in_maps


def run(inputs, **kw):
    """Build+run; returns (output [B,N,F] float32, BassKernelResults)."""
    nc = _get_nc()
    in_maps = _prep_inputs(
        inputs["x"], inputs["adj"], inputs["W0"], inputs["a_src0"],
        inputs["a_dst0"], inputs["W1"], inputs["a_src1"], inputs["a_dst1"])
    res = run_bass_kernel_spmd(nc, in_maps, list(range(NCORES)), **kw)
    outs = []
    for c in range(NCORES):
        o2 = res.results[c]["out2"].reshape(G, 4, 65, 2, N)
        po = o2[:, :, 0:F]
        den = o2[:, :, F:F + 1]
        x2 = po / den                 # already includes the 1/8 head-mean
        outs.append(x2.sum(axis=(1, 3)).transpose(0, 2, 1))  # [G, N, F]
    return np.concatenate(outs, axis=0).astype(np.float32), res


def kernel(**inputs):
    out, _ = run(inputs)
    return out


# revision 33
# speedup vs baseline: 1.1566x; 1.1566x over previous
"""BatchGAT (2-layer, 8-head GAT over 32 graphs of 512 nodes) on 8 TRN2 NeuronCores.

Data-parallel over the batch: each core processes 4 graphs, software-pipelined
over 8 (graph, layer) units. Per unit the masked-softmax attention E^T[j,i] is
built per head and aggregated with a TRANSPOSED matmul (lhsT = [hp | c-dup]
stationary, rhs = E^T moving) so each head costs 4 LDW + 4 wide MMs and the
output lands feature-major.

Head paths (softmax is invariant to per-column scaling of E; the dropped
exp(0.2 s_i) factor cancels in the softmax):
 'f' fused-DVE: ONE custom DVE op per (h, jt) [runtime-registered]:
     et = max(qbc*rsc, ed2) * adjP  with qbc = exp(0.8 s) broadcast,
     rsc = exp(d), ed2 = exp(0.2 d) per-partition scalars.
 't'/'g': dual-op TensorScalar (mult rsc, max ed2 - one DVE pass) then the
     adjP mask multiply as one full-width DVE TT ('t') or per-jt gpsimd TT
     ('g'; short Pool bursts - long ones starve concurrent DVE reads).
 'a' ACT/PE: logits s_i + adjM[j,i] via PE (K=1 ones matmul + identity
     accumulate), d_j added as the per-partition Prelu bias; et = exp(Prelu).

Normalization: c-dup lhsT columns replicate c*den across 64 PSUM rows (even
heads rows 64-127, odd heads use a flipped lhsT so den lands at rows 0-63).
Per wave: odd den reciprocal runs PSUM-direct on DVE; even den is re-based
to partitions 0-63 by one ACT copy first (DVE dst/src0 partition bases must
match; only src1 may differ - probed). Norm TT mults write x1 during PSUM
evacuation. L0 tail per wave: ACT Exp + ONE custom DVE op
x1 = min(relu(x), e^x - 1). Layer-1 folds the head-mean via c=8; po and den
ship to the host which finishes x2 = po/den and the wave/row sums.

DMA queues: bulk input loads ride the sync queue in per-jt chunks; the
qs->qd DRAM bounce issues from ACT, the qbc partition-broadcast from gpsimd
in 2-head chunks (the broadcast transfer is the unit-start latency), out2
from sync - blocking semaphore waits on a queue serialize every later DMA
issued from it, so producer-matched queues keep them wait-free.
"""

import os
import sys

if "/opt/trn_rl_repo" not in sys.path:
    sys.path.insert(0, "/opt/trn_rl_repo")

import numpy as np
import ml_dtypes

import concourse.bacc as bacc
import concourse.mybir as mybir
from concourse import tile
from concourse.bass_utils import run_bass_kernel_spmd
from concourse.alu_op_type import AluOpType

F32 = mybir.dt.float32
BF16 = mybir.dt.bfloat16
BF = ml_dtypes.bfloat16
AF = mybir.ActivationFunctionType

B, N, FIN, H, F = 32, 512, 64, 8, 64
NCORES = 8
G = B // NCORES          # graphs per core
NT = N // 128            # node tiles
C1 = H * F               # layer-1 input features (512)
BIG = 30000.0

# ---- load-balance knobs ----
# per-head path chars (index = head):
#  'a' ACT/PE path (PE logits + Prelu + Exp)
#  'f' fused custom DVE (1 op/tile)
#  't' TS-dual DVE + full-width mask TT on DVE
#  'g' TS-dual DVE + full-width mask TT on gpsimd
PATH0 = os.environ.get("GAT_P0", "afatgfta")
PATH1 = os.environ.get("GAT_P1", "afgtgata")
NORM_ENG = os.environ.get("GAT_NORM", "vector")   # vector | gpsimd | mix
ELU_ENG = os.environ.get("GAT_ELU", "vector")     # vector | split
EV_ENG = os.environ.get("GAT_EV", "act")        # act | vector | split
N_UNITS = int(os.environ.get("GAT_UNITS", "8"))
LOOK = int(os.environ.get("GAT_LOOK", "2"))
ET_BUFS = int(os.environ.get("GAT_ETB", "5"))
LR_BUFS = int(os.environ.get("GAT_LRB", "4"))
AGG_MODE = os.environ.get("GAT_AGG", "wave")      # head | wave
AGG_BUFS = int(os.environ.get("GAT_AGB", "5"))

_cached = {}
_OPS = {}


def _register_custom_ops():
    """Register the two fused DVE ops via the documented extension point
    (dve_ops.OPS); idempotent, sha computed at runtime."""
    if _OPS:
        return
    import concourse.dve_ops as dve_ops
    from concourse.dve_spec import (
        Spec, Src0, Src1, C0, C1 as SC1, maxx, minn, relu, lower,
        _has_src1,
    )
    from concourse.dve_uop import DveOpSpec

    have = {op.name for op in dve_ops.OPS}

    def mk(name, spec):
        if name in have:
            _OPS[name] = next(o for o in dve_ops.OPS if o.name == name)
            return
        row = dve_ops._CUSTOM_DVE_ROW_BASE + len(dve_ops.OPS)
        shas = {}
        for ver in ("v3", "v4"):
            uops = lower(spec, ver=ver)
            shas[ver] = DveOpSpec(
                name=name, opcode=row, uops=uops, rd1_en=_has_src1(spec)
            ).sha(ver)
        op = dve_ops.DveOp(name, spec, subdim=False, uops_sha=shas)
        dve_ops.OPS.append(op)
        dve_ops._SUB_OPCODE_FOR_NAME[name] = row
        dve_ops.CUSTOM_DVE_SPECS[name] = spec
        _OPS[name] = op

    mk("ATT_MASK_FUSED_GAT", Spec(
        body=maxx(Src0 * C0, SC1) * Src1,
        reference=lambda in0, in1, s0, s1, imm2: (
            np.maximum(in0.astype(np.float32) * s0, s1) * in1
        ).astype(np.float32),
    ))
    mk("ELU_TAIL_GAT", Spec(
        body=minn(relu(Src0), Src1 - C0),
        reference=lambda in0, in1, s0, s1, imm2: np.minimum(
            np.maximum(np.nan_to_num(in0.astype(np.float32), nan=0.0), 0.0),
            in1.astype(np.float32) - s0,
        ).astype(np.float32),
    ))


def _build():
    _register_custom_ops()
    nc = bacc.Bacc("TRN2", target_bir_lowering=False, debug=False)

    need_adjm = ("a" in PATH0) or ("a" in PATH1)
    xtb = nc.dram_tensor("xtb", [G, FIN, N], BF16, kind="ExternalInput").ap()
    adjP = nc.dram_tensor("adjP", [G, N, N], BF16, kind="ExternalInput").ap()
    adjM = nc.dram_tensor("adjM", [G, N, N], BF16, kind="ExternalInput").ap()
    ident = nc.dram_tensor("ident", [128, 128], BF16, kind="ExternalInput").ap()
    w0d = nc.dram_tensor("w0d", [FIN, F + 2 * H], BF16, kind="ExternalInput").ap()
    w1d = nc.dram_tensor("w1d", [C1, F + 2 * H], BF16, kind="ExternalInput").ap()
    out2 = nc.dram_tensor("out2", [G, 4, 65, 2 * N], F32,
                          kind="ExternalOutput").ap()

    with tile.TileContext(nc) as tc:
        _emit(nc, tc, xtb, adjP, adjM, ident, w0d, w1d, out2, need_adjm)
    nc.compile()
    return nc


def _emit(nc, tc, xtb, adjP, adjM, ident, w0d, w1d, out2, need_adjm):
    from contextlib import ExitStack

    att_op = _OPS["ATT_MASK_FUSED_GAT"]
    elu_op = _OPS["ELU_TAIL_GAT"]

    ctx = ExitStack()
    with ctx:
        wpool = ctx.enter_context(tc.tile_pool(name="weights", bufs=1))
        w0d_sb = wpool.tile([FIN, F + 2 * H], BF16, tag="w0d")
        nc.sync.dma_start(w0d_sb[:], w0d[:])
        w1d_sb = wpool.tile([128, NT, F + 2 * H], BF16, tag="w1d")
        nc.sync.dma_start(w1d_sb[:], w1d.rearrange("(c p) f -> p c f", p=128))
        id_sb = wpool.tile([128, 128], BF16, tag="ident")
        nc.sync.dma_start(id_sb[:], ident[:])
        ones1 = wpool.tile([1, 128], BF16, tag="ones1")
        nc.vector.memset(ones1[:], 1.0)

        xt_pool = ctx.enter_context(tc.tile_pool(name="xt", bufs=3))
        adj_pool = ctx.enter_context(
            tc.tile_pool(name="adj", bufs=4 if need_adjm else 2))
        row_pool = ctx.enter_context(tc.tile_pool(name="rows", bufs=3))
        sc_pool = ctx.enter_context(tc.tile_pool(name="scal", bufs=3))
        ha_pool = ctx.enter_context(tc.tile_pool(name="ha", bufs=3))
        qbc_pool = ctx.enter_context(tc.tile_pool(name="qbc", bufs=3))
        et_pool = ctx.enter_context(tc.tile_pool(name="et", bufs=ET_BUFS))
        lr_pool = ctx.enter_context(tc.tile_pool(name="lr", bufs=LR_BUFS))
        rdb_pool = ctx.enter_context(tc.tile_pool(name="rdb", bufs=2))
        x1t_pool = ctx.enter_context(tc.tile_pool(name="x1t", bufs=3))
        post_pool = ctx.enter_context(tc.tile_pool(name="post", bufs=2))
        out_pool = ctx.enter_context(tc.tile_pool(name="outs", bufs=2))
        qd_pool = ctx.enter_context(tc.tile_pool(name="qd", bufs=3, space="DRAM"))

        ps_agg = ctx.enter_context(tc.tile_pool(
            name="ps_agg", bufs=AGG_BUFS if AGG_MODE == "head" else 2,
            space="PSUM"))
        ps_pp = ctx.enter_context(tc.tile_pool(name="ps_pp", bufs=1, space="PSUM"))
        ps_sd = ctx.enter_context(tc.tile_pool(name="ps_sd", bufs=1, space="PSUM"))
        ps_lg = ctx.enter_context(tc.tile_pool(
            name="ps_lg", bufs=1 if AGG_MODE == "head" else 2, space="PSUM"))

        graphs = {}

        def prologue(g, layer):
            """Projection + row/scalar extraction + broadcasts for one unit."""
            st = {}
            path = PATH0 if layer == 0 else PATH1
            if layer == 0:
                # inputs ride the tensor queue (no waits; frees q1 for qbc)
                xt = xt_pool.tile([FIN, N], BF16, tag="xt", name=f"xt_{g}")
                nc.sync.dma_start(xt[:], xtb[g])
                # per-jt chunk DMAs parallelize across DMA engines (a single
                # 512KB transfer rides one ~22.5GB/s engine for ~23us)
                ap_ = adj_pool.tile([128, NT * N], BF16, tag="adjp",
                                    name=f"adjp_{g}")
                apv = adjP[g].rearrange("(j p) i -> p j i", p=128)
                for jt in range(NT):
                    nc.sync.dma_start(ap_[:, jt * N:(jt + 1) * N],
                                      apv[:, jt])
                gd = dict(xt=xt, adjp=ap_)
                if need_adjm:
                    am_ = adj_pool.tile([128, NT * N], BF16, tag="adjm",
                                        name=f"adjm_{g}")
                    amv = adjM[g].rearrange("(j p) i -> p j i", p=128)
                    for jt in range(NT):
                        nc.sync.dma_start(am_[:, jt * N:(jt + 1) * N],
                                          amv[:, jt])
                    gd["adjm"] = am_
                graphs[g] = gd
            gs = graphs[g]
            xt, x1t = gs["xt"], gs.get("x1t")

            # s, d row vectors first: they gate the qs->qd->qbc DMA chain
            psd = ps_sd.tile([2 * H, N], F32, tag="sd", name=f"psd_{g}_{layer}")
            if layer == 0:
                nc.tensor.matmul(psd[:], w0d_sb[:, F:], xt[:],
                                 start=True, stop=True)
            else:
                for ct in range(NT):
                    nc.tensor.matmul(psd[:], w1d_sb[:, ct, F:],
                                     x1t[:, ct * N:(ct + 1) * N],
                                     start=(ct == 0), stop=(ct == NT - 1))

            # ---- projections: pp_all[j, jt*80: [hp(64) | d(8) | s(8)]] ----
            pp = ps_pp.tile([128, NT * (F + 2 * H)], F32, tag="pp",
                            name=f"pp_{g}_{layer}")
            W = F + 2 * H
            for jt in range(NT):
                dst = pp[:, jt * W:(jt + 1) * W]
                if layer == 0:
                    nc.tensor.matmul(dst, xt[:, jt * 128:(jt + 1) * 128],
                                     w0d_sb[:], start=True, stop=True)
                else:
                    for ct in range(NT):
                        nc.tensor.matmul(
                            dst, x1t[:, ct * N + jt * 128:ct * N + (jt + 1) * 128],
                            w1d_sb[:, ct, :], start=(ct == 0), stop=(ct == NT - 1))

            # ---- ACT extractions (read all 16 psd rows: ACT srcs must be
            # 32-aligned; rows 0-7 are d-junk, the DMA slices rows 8-15) ----
            qs = row_pool.tile([2 * H, 2 * N], BF16, tag="qs",
                               name=f"qs_{g}_{layer}")
            nc.scalar.activation(qs[:, 0:N], psd[:], AF.Exp, scale=0.8)
            if "a" in path:
                nc.scalar.activation(qs[:, N:2 * N], psd[:], AF.Copy)
            ppv = pp[:].rearrange("p (j c) -> p j c", j=NT, c=W)
            # per-partition scalars from the d columns: r=exp(d), ed2=exp(.2 d)
            sc = sc_pool.tile([128, 3 * NT * H], F32, tag="sc",
                              name=f"sc_{g}_{layer}")
            scv = sc[:].rearrange("p (k j h) -> p k j h", k=3, j=NT)
            nc.scalar.activation(scv[:, 0], ppv[:, :, F:F + H], AF.Exp)
            nc.scalar.activation(scv[:, 1], ppv[:, :, F:F + H], AF.Exp, scale=0.2)
            if "a" in path:
                nc.scalar.activation(scv[:, 2], ppv[:, :, F:F + H], AF.Copy)
            rsc = scv[:, 0]     # [128, jt, h]
            ed2 = scv[:, 1]
            dcol = scv[:, 2]

            # lhsT tiles: ha = [hp | c-dup]; haf = [c-dup | hp] (layer 0 only,
            # for odd heads so their po lands at rows 64-127)  (c=1 or 8)
            cval = 1.0 if layer == 0 else 8.0
            ha = ha_pool.tile([128, NT * 128], BF16, tag="ha",
                              name=f"ha_{g}_{layer}")
            hav = ha[:].rearrange("p (j c) -> p j c", j=NT)
            nc.scalar.activation(hav[:, :, 0:F], ppv[:, :, 0:F], AF.Copy)
            nc.vector.memset(hav[:, :, F:128], cval)
            haf = None
            if layer == 0:
                haf = ha_pool.tile([128, NT * 128], BF16, tag="haf",
                                   name=f"haf_{g}_{layer}")
                hfv = haf[:].rearrange("p (j c) -> p j c", j=NT)
                nc.scalar.activation(hfv[:, :, F:128], ppv[:, :, 0:F], AF.Copy)
                nc.vector.memset(hfv[:, :, 0:F], cval)

            # q broadcast via DRAM bounce; s gathered flat for A-path matmuls.
            # Issue these on scalar/gpsimd queues: their semaphore waits would
            # otherwise serialize all later input loads behind the bounce.
            qd = qd_pool.tile([H, 2 * N], BF16, tag="qd", name=f"qd_{g}_{layer}")
            nc.scalar.dma_start(qd[:], qs[H:2 * H, :])
            qbc = qbc_pool.tile([128, H * N], BF16, tag="qbc",
                                name=f"qbc_{g}_{layer}")
            qbcv = qbc[:].rearrange("p (h i) -> p h i", h=H)
            for hc in range(0, H, 2):
                nc.gpsimd.dma_start(
                    qbcv[:, hc:hc + 2],
                    qd[hc:hc + 2, 0:N].unsqueeze(0).partition_broadcast(128))
            sfl = None
            if "a" in path:
                sfl = row_pool.tile([1, H * N], BF16, tag="sfl",
                                    name=f"sfl_{g}_{layer}")
                nc.gpsimd.dma_start(sfl[:], qd[:, N:2 * N])

            st.update(ha=ha, haf=haf, qbc=qbc, sfl=sfl, rsc=rsc, ed2=ed2,
                      dcol=dcol)
            return st

        def main(g, layer, st):
            gs = graphs[g]
            path = PATH0 if layer == 0 else PATH1
            adjp, adjm = gs["adjp"], gs.get("adjm")
            ha, haf, qbc, sfl = st["ha"], st["haf"], st["qbc"], st["sfl"]
            rsc, ed2, dcol = st["rsc"], st["ed2"], st["dcol"]

            if layer == 0:
                x1t = x1t_pool.tile([128, NT * N], BF16, tag="x1t",
                                    name=f"x1t_{g}")
                graphs[g]["x1t"] = x1t
                dstn = x1t_pool.tile([128, NT * N], BF16, tag="x1n",
                                     name=f"x1n_{g}")
            else:
                dstn = None

            for w in range(4):           # four waves of 2 heads
                if AGG_MODE != "head":
                    agg = ps_agg.tile([128, 2 * N], F32, tag="agg",
                                      name=f"agg_{g}_{layer}_{w}")
                for hh in range(2):
                    h = 2 * w + hh
                    p = path[h]
                    if AGG_MODE == "head":
                        aggh = ps_agg.tile([128, N], F32, tag="agg",
                                           name=f"agg_{g}_{layer}_{h}")
                    et = et_pool.tile([128, NT * N], BF16, tag="et",
                                      name=f"et_{g}_{layer}_{h}")
                    if p == "a":
                        lr = lr_pool.tile([128, NT * N], BF16, tag="lr",
                                          name=f"lr_{g}_{layer}_{h}")
                        for jt in range(NT):
                            lg = ps_lg.tile([128, N], F32, tag="lg",
                                            name=f"lg_{g}_{layer}_{h}_{jt}")
                            nc.tensor.matmul(
                                lg[:], ones1[:], sfl[:, h * N:(h + 1) * N],
                                start=True, stop=False)
                            nc.tensor.matmul(
                                lg[:], id_sb[:],
                                adjm[:, jt * N:(jt + 1) * N],
                                start=False, stop=True)
                            nc.scalar.activation(
                                lr[:, jt * N:(jt + 1) * N], lg[:], AF.Prelu,
                                bias=dcol[:, jt, h:h + 1], alpha=0.2)
                        nc.scalar.activation(et[:], lr[:], AF.Exp)
                    elif p == "f":
                        for jt in range(NT):
                            nc.vector._custom_dve(
                                att_op,
                                out=et[:, jt * N:(jt + 1) * N],
                                in0=qbc[:, h * N:(h + 1) * N],
                                in1=adjp[:, jt * N:(jt + 1) * N],
                                s0=rsc[:, jt, h:h + 1],
                                s1=ed2[:, jt, h:h + 1])
                    else:
                        w_ = lr_pool.tile([128, NT * N], BF16, tag="lr",
                                          name=f"w_{g}_{layer}_{h}")
                        for jt in range(NT):
                            nc.vector.tensor_scalar(
                                w_[:, jt * N:(jt + 1) * N],
                                qbc[:, h * N:(h + 1) * N],
                                rsc[:, jt, h:h + 1], ed2[:, jt, h:h + 1],
                                AluOpType.mult, AluOpType.max)
                        if p == "g":
                            # per-jt chunks: short gpsimd bursts contend far
                            # less with concurrent DVE work than full-width
                            for jt in range(NT):
                                nc.gpsimd.tensor_tensor(
                                    et[:, jt * N:(jt + 1) * N],
                                    w_[:, jt * N:(jt + 1) * N],
                                    adjp[:, jt * N:(jt + 1) * N],
                                    AluOpType.mult)
                        else:
                            nc.vector.tensor_tensor(et[:], w_[:], adjp[:],
                                                    AluOpType.mult)
                    lhs = ha if (layer == 1 or h % 2 == 0) else haf
                    adst = aggh[:] if AGG_MODE == "head" else \
                        agg[:, hh * N:(hh + 1) * N]
                    for jt in range(NT):
                        nc.tensor.matmul(
                            adst,
                            lhs[:, jt * 128:(jt + 1) * 128],
                            et[:, jt * N:(jt + 1) * N],
                            start=(jt == 0), stop=(jt == NT - 1))

                    # ---- per-head evacuation (head mode) ----
                    if AGG_MODE != "head":
                        continue
                    if layer == 0:
                        dv = dstn[:, w * N:(w + 1) * N]
                        rdh = rdb_pool.tile([64, N], F32, tag="rdb",
                                            name=f"rd_{g}_{layer}_{h}")
                        if h % 2 == 0:
                            # ha: po rows 0-63, den rows 64-127 (ACT re-base)
                            dnE = rdb_pool.tile([64, N], F32, tag="dnE",
                                                name=f"dnE_{g}_{layer}_{h}")
                            nc.scalar.activation(dnE[:], aggh[64:128, :],
                                                 AF.Copy)
                            nc.vector.reciprocal_approx_fast(rdh[:], dnE[:])
                            nc.vector.tensor_tensor(dv[0:64], aggh[0:64, :],
                                                    rdh[:], AluOpType.mult)
                        else:
                            # haf: den rows 0-63 (PSUM-direct), po rows 64-127
                            nc.vector.reciprocal_approx_fast(rdh[:],
                                                             aggh[0:64, :])
                            nc.vector.tensor_tensor(dv[64:128],
                                                    aggh[64:128, :],
                                                    rdh[:], AluOpType.mult)
                    else:
                        ev = out_pool.tile([65, N], F32, tag="ev",
                                           name=f"ev_{g}_{h}")
                        if EV_ENG == "act" or (EV_ENG == "split"
                                               and h % 2 == 0):
                            nc.scalar.activation(ev[:], aggh[0:65, :],
                                                 AF.Copy)
                        else:
                            nc.vector.tensor_copy(ev[:], aggh[0:65, :])
                        nc.sync.dma_start(
                            out2[g, w, :, hh * N:(hh + 1) * N], ev[:])

                # ---- evacuate wave (wave mode) ----
                if AGG_MODE == "head":
                    continue
                if layer == 0:
                    # dens: odd head rows 0-63 cols N:2N (PSUM-direct recip);
                    # even head rows 64-127 cols 0:N (ACT copy re-bases to
                    # partitions 0-63, then recip). DVE partition rule: dst
                    # and src0 bases match; src1 base may sit lower (proven).
                    rd = rdb_pool.tile([64, 2 * N], F32, tag="rdb",
                                       name=f"rdb_{g}_{layer}_{w}")
                    dnE = rdb_pool.tile([64, N], F32, tag="dnE",
                                        name=f"dnE_{g}_{layer}_{w}")
                    nc.scalar.activation(dnE[:], agg[64:128, 0:N], AF.Copy)
                    nc.vector.reciprocal_approx_fast(rd[:, 0:N],
                                                     agg[0:64, N:2 * N])
                    nc.vector.reciprocal_approx_fast(rd[:, N:2 * N], dnE[:])
                    dv = dstn[:, w * N:(w + 1) * N]
                    ne = (nc.gpsimd if NORM_ENG == "gpsimd" else nc.vector)
                    ne2 = (nc.gpsimd if NORM_ENG in ("gpsimd", "mix")
                           else nc.vector)
                    ne.tensor_tensor(dv[0:64], agg[0:64, 0:N],
                                     rd[:, N:2 * N], AluOpType.mult)
                    ne2.tensor_tensor(dv[64:128], agg[64:128, N:2 * N],
                                      rd[:, 0:N], AluOpType.mult)
                else:
                    # L1: ship po rows 0-63 + den row 64 out; host normalizes
                    ev = out_pool.tile([65, 2 * N], F32, tag="ev",
                                       name=f"ev_{g}_{w}")
                    if EV_ENG == "act":
                        nc.scalar.activation(ev[:], agg[0:65, :], AF.Copy)
                    elif EV_ENG == "vector":
                        nc.vector.tensor_copy(ev[:], agg[0:65, :])
                    else:
                        nc.scalar.activation(ev[:, 0:N], agg[0:65, 0:N],
                                             AF.Copy)
                        nc.vector.tensor_copy(ev[:, N:2 * N],
                                              agg[0:65, N:2 * N])
                    nc.sync.dma_start(out2[g, w, :, 0:N], ev[:, 0:N])
                    nc.sync.dma_start(out2[g, w, :, N:2 * N], ev[:, N:2 * N])

            if layer == 0:
                # per-wave elu tail: elu(x) = min(relu(x), exp(x) - 1);
                # chunked so x1t columns unlock as each wave's dv lands
                x1t = graphs[g]["x1t"]
                expt = post_pool.tile([128, NT * N], BF16, tag="expt",
                                      name=f"expt_{g}")
                for w in range(4):
                    sl = slice(w * N, (w + 1) * N)
                    nc.scalar.activation(expt[:, sl], dstn[:, sl], AF.Exp)
                    nc.vector._custom_dve(elu_op, out=x1t[:, sl],
                                          in0=dstn[:, sl],
                                          in1=expt[:, sl], s0=1.0)

        # software-pipelined unit order: prologue of unit k+1 lands before
        # main of unit k
        if os.environ.get("GAT_ORDER", "mix") == "l0first":
            U = [(0, 0), (1, 0), (2, 0), (3, 0), (0, 1), (1, 1), (2, 1), (3, 1)]
        else:
            U = [(0, 0), (1, 0), (2, 0), (0, 1), (3, 0), (1, 1), (2, 1), (3, 1)]
        U = U[:N_UNITS]
        pending = {}
        for k in range(min(LOOK, len(U))):
            pending[U[k]] = prologue(*U[k])
        for i, u in enumerate(U):
            main(u[0], u[1], pending.pop(u))
            if i + LOOK < len(U):
                nxt = U[i + LOOK]
                pending[nxt] = prologue(*nxt)
        if not any(l == 1 for _, l in U):
            # bisect mode: make sure out2 is written so fetch succeeds
            z = out_pool.tile([65, 2 * N], F32, tag="ev", name="zz")
            nc.vector.memset(z[:], 0.0)
            for g in range(G):
                for w in range(4):
                    nc.sync.dma_start(out2[g, w], z[:])


def _get_nc():
    if "nc" not in _cached:
        _cached["nc"] = _build()
    return _cached["nc"]


def _prep_inputs(x, adj, W0, a_src0, a_dst0, W1, a_src1, a_dst1):
    x = np.asarray(x, np.float32)
    adj = np.array(adj, np.float32, copy=True)
    idx = np.arange(N)
    adj[:, idx, idx] = 1.0  # self loops (reference mutates adj the same way)
    xT = np.ascontiguousarray(x.transpose(0, 2, 1)).astype(BF)  # [B, 64, 512]
    adjPf = np.where(adj > 0, np.float32(1), np.float32(0)).astype(BF)
    adjMf = np.where(adj > 0, np.float32(0), np.float32(-BIG)).astype(BF)
    identf = np.eye(128, dtype=np.float32).astype(BF)
    W0 = np.asarray(W0, np.float32)
    W1 = np.asarray(W1, np.float32)
    w0d = np.concatenate(
        [W0, W0 @ np.asarray(a_dst0, np.float32),
         W0 @ np.asarray(a_src0, np.float32)], axis=1).astype(BF)
    w1d = np.concatenate(
        [W1, W1 @ np.asarray(a_dst1, np.float32),
         W1 @ np.asarray(a_src1, np.float32)], axis=1).astype(BF)
    in_maps = []
    for c in range(NCORES):
        sl = slice(c * G, (c + 1) * G)
        in_maps.append(dict(
            xtb=np.ascontiguousarray(xT[sl]),
            adjP=np.ascontiguousarray(adjPf[sl]),
            adjM=np.ascontiguousarray(adjMf[sl]),
            ident=identf, w0d=w0d, w1d=w1d,
        ))
    return in_maps


def run(inputs, **kw):
    """Build+run; returns (output [B,N,F] float32, BassKernelResults)."""
    nc = _get_nc()
    in_maps = _prep_inputs(
        inputs["x"], inputs["adj"], inputs["W0"], inputs["a_src0"],
        inputs["a_dst0"], inputs["W1"], inputs["a_src1"], inputs["a_dst1"])
    res = run_bass_kernel_spmd(nc, in_maps, list(range(NCORES)), **kw)
    outs = []
    for c in range(NCORES):
        o2 = res.results[c]["out2"].reshape(G, 4, 65, 2, N)
        po = o2[:, :, 0:F]
        den = o2[:, :, F:F + 1]
        x2 = po / den                 # already includes the 1/8 head-mean
        outs.append(x2.sum(axis=(1, 3)).transpose(0, 2, 1))  # [G, N, F]
    return np.concatenate(outs, axis=0).astype(np.float32), res


def kernel(**inputs):
    out, _ = run(inputs)
    return out
